# revision 9
# baseline (speedup 1.0000x reference)
"""KAN feed-forward on Trainium2 — Bass/Tile kernel, 8-core data-parallel.

Math transform: each KAN layer is
    y = silu(x) @ scale_base + einsum('nig,iog,io->no', B(x), coef, scale_sp)
with B the (G=5, K=3) uniform-grid B-spline basis (8 funcs/dim, u = 2.5x+5.5).
All 8 basis functions are shifts of the cardinal cubic B-spline:
    B_g(u) = b3(u - g),  b3(t) = (1/6)[relu(2-|t-2|)^3 - 4*relu(1-|t-2|)^3]
Evaluating B_g DIRECTLY (values in [0, 2/3]) instead of via truncated powers
keeps the feature matrix perfectly conditioned, so fp16 matmul operands are
numerically safe (truncated powers reach 1331 and cancel to O(1) -> fp16
catastrophically fails).  fp16 operands stream the PE at 1 row/cycle (4x the
fp32 rate) and the 9 features/dim (vs 12 for powers) cut PE rows 25% more.

Feature chain per basis g (s = |2.5x + 5.5 - (g+2)|, emitted from x in ONE
ACT op; B-splines vanish outside [0,11] so no clamping is needed):
    an  = min(s,2)-2 = -relu(2-s)       [DVE tensor_scalar, 4x mode]
    bn  = min(an,-1)+1 = -relu(1-s)     [DVE tensor_scalar]
    a2q = Square(1-s/2) = relu(2-s)^2/4 [ACT]   b2 = bn*bn [DVE/Pool mul]
    a3q = a2q*an = -relu(2-s)^3/4       b3 = b2*bn = -relu(1-s)^3
    F   = b3 - a3q = 1.5*B_g >= 0       [plain tensor_tensor subtract]
(walrus only allows TensorTensor/copies on GPSIMD, so the x4 stencil factor
rides the ACT Square's scale and the combine is a plain subtract.)  2/3 and
scale_sp fold into host-packed fp16 weights.  All ACT functions used
(Silu/Abs/Square) live in one activation table set -> no table switches.

Per-core layout (512 tokens/core):
  L1: out1[h_blk, tok] over 36 K-tiles (4 silu + 32 spline), 8 PSUM banks.
  L2: h copied PSUM->SBUF as fp16 (frees banks for the aliased L2 PSUM),
      out2[tok_blk, o] over 72 K-tiles (8 silu + 64 spline), 4 PSUM banks.
"""

import sys
from contextlib import ExitStack

import numpy as np

for _p in ("/opt/trn_rl_repo",):
    if _p not in sys.path:
        sys.path.insert(0, _p)

# ---------------------------------------------------------------- constants
NB = 8  # B-spline basis functions per input dim
D, H, O = 512, 1024, 512
NCORES = 8
NTOK = 4096
TOK = NTOK // NCORES  # 512 tokens per core
P = 128

L1_NK = 4 * (1 + NB)  # 36 K-tiles of 128 (silu + 8 spline per i-block)
L2_NK = 8 * (1 + NB)  # 72 K-tiles of 128

_BUILD_CACHE: dict = {}


# ---------------------------------------------------------------- host prep
def _pack_w1(coef1, scale_sp1, scale_base1) -> np.ndarray:
    """-> (36, 128, 1024) fp16; k = ib*9 is the silu tile, +1+g the splines."""
    A = (coef1.astype(np.float64) * scale_sp1.astype(np.float64)[:, :, None]) * (4.0 / 6.0)
    w1 = np.empty((L1_NK, P, H), np.float16)
    for ib in range(4):
        rows = slice(ib * P, (ib + 1) * P)
        w1[ib * 9] = scale_base1[rows].astype(np.float16)
        for g in range(NB):
            w1[ib * 9 + 1 + g] = A[rows, :, g].astype(np.float16)
    return np.ascontiguousarray(w1)


def _pack_w2(coef2, scale_sp2, scale_base2) -> np.ndarray:
    """-> (72, 128, 512) fp16."""
    A = (coef2.astype(np.float64) * scale_sp2.astype(np.float64)[:, :, None]) * (4.0 / 6.0)
    w2 = np.empty((L2_NK, P, O), np.float16)
    for j in range(8):
        rows = slice(j * P, (j + 1) * P)
        w2[j * 9] = scale_base2[rows].astype(np.float16)
        for g in range(NB):
            w2[j * 9 + 1 + g] = A[rows, :, g].astype(np.float16)
    return np.ascontiguousarray(w2)


# ---------------------------------------------------------------- bass build
def _build_kernel():
    if "nc" in _BUILD_CACHE:
        return _BUILD_CACHE["nc"]

    import concourse.mybir as mybir
    import concourse.tile as tile
    from concourse import bacc

    AF = mybir.ActivationFunctionType
    OP = mybir.AluOpType
    F32 = mybir.dt.float32
    F16 = mybir.dt.float16

    nc = bacc.Bacc("TRN2", target_bir_lowering=False, debug=False, num_devices=NCORES)

    xT = nc.dram_tensor("xT", (D, TOK), F32, kind="ExternalInput").ap()
    w1 = nc.dram_tensor("w1", (L1_NK, P, H), F16, kind="ExternalInput").ap()
    w2 = nc.dram_tensor("w2", (L2_NK, P, O), F16, kind="ExternalInput").ap()
    out = nc.dram_tensor("out", (TOK, O), F32, kind="ExternalOutput").ap()

    with tile.TileContext(nc) as tc, ExitStack() as ctx:
        persist = ctx.enter_context(tc.tile_pool(name="persist", bufs=1))
        sp = ctx.enter_context(tc.tile_pool(name="sp", bufs=4))
        cp = ctx.enter_context(tc.tile_pool(name="cp", bufs=4))
        fp = ctx.enter_context(tc.tile_pool(name="fp", bufs=4))
        w1p = ctx.enter_context(tc.tile_pool(name="w1p", bufs=4))
        w2p = ctx.enter_context(tc.tile_pool(name="w2p", bufs=6))
        outp = ctx.enter_context(tc.tile_pool(name="outp", bufs=2))
        psum = ctx.enter_context(tc.tile_pool(name="psum", bufs=1, space="PSUM"))

        _bias_cache: dict = {}

        def bias_ap(val: float):
            if val not in _bias_cache:
                t = persist.tile([P, 1], F32, tag=f"bias{len(_bias_cache)}",
                                 name=f"bias_{len(_bias_cache)}")
                nc.vector.memset(t, val)
                _bias_cache[val] = t
            return _bias_cache[val]

        # Pool (GPSIMD) only runs TensorTensor/copies under walrus; it is also
        # ~3.4x slower per tile than DVE's 2x fp16 mode, so it takes ~5/18 of
        # the tensor_tensor stream and DVE keeps the rest.
        tt_idx = [0]

        def tt_mul(out_t, in0, in1):
            eng = nc.gpsimd if (tt_idx[0] % 18) < 5 else nc.vector
            tt_idx[0] += 1
            eng.tensor_mul(out_t, in0, in1)

        def tt_sub(out_t, in0, in1):
            eng = nc.gpsimd if (tt_idx[0] % 18) < 5 else nc.vector
            tt_idx[0] += 1
            eng.tensor_tensor(out_t, in0, in1, OP.subtract)

        def emit_feature(src_ap, g, name):
            """F = 1.5*B_g(2.5*src + 5.5) as a [P, TOK] fp16 tile."""
            s = sp.tile([P, TOK], F16, tag="s", name=f"s{name}")
            nc.scalar.activation(s, src_ap, AF.Abs,
                                 bias=bias_ap(5.5 - (g + 2.0)), scale=2.5)
            an = cp.tile([P, TOK], F16, tag="an", name=f"an{name}")
            nc.vector.tensor_scalar(an, s, 2.0, 2.0, OP.min, OP.subtract)
            bn = cp.tile([P, TOK], F16, tag="bn", name=f"bn{name}")
            nc.vector.tensor_scalar(bn, an, -1.0, -1.0, OP.min, OP.subtract)
            a2q = cp.tile([P, TOK], F16, tag="a2q", name=f"a2q{name}")
            nc.scalar.activation(a2q, s, AF.Square, bias=bias_ap(1.0), scale=-0.5)
            b2 = cp.tile([P, TOK], F16, tag="b2", name=f"b2{name}")
            tt_mul(b2, bn, bn)
            a3q = cp.tile([P, TOK], F16, tag="a3q", name=f"a3q{name}")
            tt_mul(a3q, a2q, an)
            b3 = cp.tile([P, TOK], F16, tag="b3", name=f"b3{name}")
            tt_mul(b3, b2, bn)
            F = fp.tile([P, TOK], F16, tag="f", name=f"F{name}")
            tt_sub(F, b3, a3q)
            return F

        # ---- L1 inputs ---------------------------------------------------
        xt = []
        for ib in range(4):
            t = persist.tile([P, TOK], F32, tag=f"xt{ib}", name=f"xt{ib}")
            nc.sync.dma_start(out=t, in_=xT[ib * P : (ib + 1) * P, :])
            xt.append(t)

        # ---- L1: out1[h_blk, tok] over 36 K-tiles ------------------------
        pb = [psum.tile([P, TOK], F32, tag=f"p{ob}", name=f"p{ob}")
              for ob in range(8)]

        for k in range(L1_NK):
            ib, t = divmod(k, 1 + NB)
            if t == 0:
                rhs = persist.tile([P, TOK], F16, tag=f"si1{ib}", name=f"si1_{ib}")
                nc.scalar.activation(rhs, xt[ib], AF.Silu, bias=bias_ap(0.0))
            else:
                rhs = emit_feature(xt[ib], t - 1, f"a{ib}g{t-1}")
            wt = w1p.tile([P, H], F16, tag="w1k", name=f"w1k{k}")
            nc.sync.dma_start(out=wt, in_=w1[k])
            for ob in range(8):
                nc.tensor.matmul(pb[ob], wt[:, ob * P : (ob + 1) * P], rhs,
                                 start=(k == 0), stop=(k == L1_NK - 1))

        # ---- boundary: h -> SBUF fp16 (frees pb[0..3] for L2's PSUM) -----
        hb = []
        for j in range(8):
            ht = persist.tile([P, TOK], F16, tag=f"h{j}", name=f"h{j}")
            nc.vector.tensor_copy(ht, pb[j])
            hb.append(ht)

        # ---- L2: out2[tok_blk, o] over 72 K-tiles ------------------------
        qb = [psum.tile([P, O], F32, tag=f"p{tb}", name=f"q{tb}")
              for tb in range(4)]

        for k in range(L2_NK):
            j, t = divmod(k, 1 + NB)
            if t == 0:
                lhsT = persist.tile([P, TOK], F16, tag=f"si2{j}", name=f"si2_{j}")
                nc.scalar.activation(lhsT, hb[j], AF.Silu, bias=bias_ap(0.0))
            else:
                lhsT = emit_feature(hb[j], t - 1, f"b{j}g{t-1}")
            wt = w2p.tile([P, O], F16, tag="w2k", name=f"w2k{k}")
            nc.sync.dma_start(out=wt, in_=w2[k])
            for tb in range(4):
                nc.tensor.matmul(qb[tb], lhsT[:, tb * P : (tb + 1) * P], wt,
                                 start=(k == 0), stop=(k == L2_NK - 1))

        # ---- store -------------------------------------------------------
        for tb in range(4):
            ot = outp.tile([P, O], F32, tag="ot", name=f"ot{tb}")
            nc.vector.tensor_copy(ot, qb[tb])
            nc.sync.dma_start(out=out[tb * P : (tb + 1) * P, :], in_=ot)

    nc.compile()
    _BUILD_CACHE["nc"] = nc
    return nc


# ---------------------------------------------------------------- entry
def kernel(x, coef1, scale_base1, scale_sp1, coef2, scale_base2, scale_sp2,
           _want_trace=False):
    from concourse.bass_utils import run_bass_kernel_spmd

    x_flat = np.asarray(x, np.float32).reshape(NTOK, D)
    w1 = _pack_w1(np.asarray(coef1), np.asarray(scale_sp1), np.asarray(scale_base1))
    w2 = _pack_w2(np.asarray(coef2), np.asarray(scale_sp2), np.asarray(scale_base2))

    nc = _build_kernel()

    in_maps = []
    for c in range(NCORES):
        xs = x_flat[c * TOK : (c + 1) * TOK]  # (TOK, D)
        in_maps.append(
            {
                "xT": np.ascontiguousarray(xs.T),
                "w1": w1,
                "w2": w2,
            }
        )

    res = run_bass_kernel_spmd(
        nc, in_maps, core_ids=list(range(NCORES)), trace=_want_trace
    )
    outs = [res.results[c]["out"] for c in range(NCORES)]
    full = np.concatenate(outs, axis=0).reshape(x.shape[0], x.shape[1], O)
    if _want_trace:
        kernel._last_results = res  # stash for test harness profiling
    return full.astype(np.float32)
